# revision 53
# baseline (speedup 1.0000x reference)
"""Trainium2 Bass kernel for nn_DirectionalWedgeBias.

Computes, per (batch b, head h):
    v      = x[b].reshape(T, H, Dh)[:, h, :]          # [T, Dh]
    v_hat  = v / max(||v||_2, eps)  (row-wise)
    S      = A[h] - A[h]^T                            # [Dh, Dh]
    wedge  = (v_hat @ S) @ v_hat^T                    # [T, T]

Full shapes: x [2, 2048, 1024] f32, A [16, 64, 64] f32 -> out [2, 16, 2048, 2048] f32.

Sharding: 32 independent (b, h) pairs split 4-per-core across 8 NeuronCores
(data + head parallel; the tiny skew-symmetric S is replicated/sliced with the
heads). Host pre-slices x into per-core [4, T, Dh] blocks, forms S = A - A^T,
and re-stacks the per-core [4, T, T] results.

Per-core dataflow (Tile framework), fp16 end to end (rel err ~5e-4 vs the
2e-2 gate; f32 is kept only for the row norms). The kernel is bound by
PSUM->SBUF evacuation: matmuls may only write f32 PSUM, and only ACT and DVE
may read PSUM (the BIR verifier rejects GPSIMD PSUM access), so those two
engines are reserved for evacuation and everything else is pushed elsewhere:
  - x loads use a 2-rows-per-partition interleave (t = n*256 + 2*part + r) so
    each partition receives contiguous 512 B runs (full-rate descriptors
    instead of the <512 B read-modify-write class)
  - row-normalize per 512-row group: square + tree-reduce + v_hat multiply
    run on Pool/gpsimd (SBUF-only ops are legal there; gpsimd cannot reduce
    the free axis so the reduction is a log2 tensor_add tree), ACT does the
    sqrt, DVE only the tiny reciprocal; v_hat is written as fp16
  - PE-transpose v_hat chunks into vt [64, T] fp16, kept in native block
    order: the transpose PSUM tile is fp16, so the straight (non-scattering)
    copy qualifies for DVE's 2x half-word mode; the wedge rhs un-permutes
    via a strided read at no modeled cost. SvT = S @ vT (block-order rhs)
    lands block-ordered, so wedge lhsT slices stay contiguous
  - wedge: two fp16 matmuls per [128, 1024] PSUM tile in a 3-deep ring (6 of
    the 8 banks; 1 bank each for the transpose and SvT tiles); ACT and DVE
    strictly alternate ring slots so the ring never waits twice on one
    queue, and each [128, 1024] chunk is cast to fp16 into the staging tile
  - stores are fp16 (half the f32 DMA bytes) and each chunk is its own DMA
    (free in the cost model -- DMA cost is purely bytes-proportional above
    the 500 ns floor), load-balanced over SP and Pool, with a one-chunk
    deferral per queue so a store whose evacuation lags never head-of-line
    blocks the queue; the host widens fp16 back to f32
  - a greedy balancer assigns every evacuation/store/load by modeled ns cost
  - walrus encodes at most ONE semaphore wait on most instructions, so
    `_spill_waits` post-processes the Tile-scheduled BIR, hoisting excess
    waits onto preceding same-engine EventSemaphores

Cost-model (CoreSim) per-core time: ~95.3 us (engine busy: DVE ~84, ACT ~83,
SP ~64, Pool ~55, PE ~63 -- evacuation-bound; the legal floor for this
dataflow is ~93 us). Verified end to end through neuronxcc + the 8-core axon
run at rel err 4.6e-4.
"""

import numpy as np

B = 2
T = 2048
D = 1024
H = 16
Dh = 64
N_CORES = 8
PAIRS = (B * H) // N_CORES  # 4 per core
P = 128  # SBUF partitions
NB = T // 256  # 8 n-blocks (256 rows) per pair
R = 2  # t-rows per partition within an n-block

_COMPILED = {}

# test-harness knobs (default off; harness calls kernel() with these untouched)
TRACE = False
LAST_RESULT = None

# modeled per-instruction costs (ns) for the greedy engine balancer.
# Only ACT and DVE can read PSUM (the BIR verifier rejects GPSIMD PSUM
# access), so they evacuate; Pool does the SBUF-side norm math and shares
# stores/loads with SP.
_EVAC_1024 = {"ACT": 1100.0, "DVE": 1260.0}
_VT_EVAC = {"ACT": 1100.0, "DVE": 720.0}  # fp16 PSUM src: DVE gets 2x mode
_EVAC_512 = {"ACT": 670.0, "DVE": 900.0}
_STORE_Q = 850.0  # [128, 1024] fp16 chunk = 2048 B/partition
_LOAD_PAIR = 1729.0
_NORM_GROUP = 490.0  # vsq + reduce per 512-row group ([128, 4, 64]) on Pool
_VHAT_GROUP = 270.0
MAX_CONSEC = {"ACT": 1, "DVE": 1}  # max consecutive ring slots per engine


def _build_nc(pairs=PAIRS, t=T, spill=True):
    _import_concourse()
    from contextlib import ExitStack

    import concourse.bass as bass
    import concourse.tile as tile
    from concourse import mybir

    f32 = mybir.dt.float32
    f16 = mybir.dt.float16
    nb = t // 256  # n-blocks per pair
    ng = t // 512  # 512-row load/norm groups per pair

    nc = bass.Bass()
    x_in = nc.declare_dram_parameter("x", [pairs, t, Dh], f32, isOutput=False)
    s_in = nc.declare_dram_parameter("s", [pairs, Dh, Dh], f32, isOutput=False)
    id_in = nc.declare_dram_parameter("ident", [P, P], f32, isOutput=False)
    out_d = nc.declare_dram_parameter("out", [pairs, t, t], f16, isOutput=True)

    busy = {"ACT": 0.0, "DVE": 0.0, "POOL": 0.0, "SP": 0.0}
    last_ev = {"e": None, "n": 0}

    def pick(cands, costs):
        k = min(cands, key=lambda e: busy[e] + costs[e])
        busy[k] += costs[k]
        return k

    def pick_ev(costs):
        # balanced ACT/DVE evac choice; cap consecutive ring slots per engine
        # (ACT is the faster evacuator so it may take short bursts, DVE never
        # repeats) so the PSUM ring always drains through both queues
        cands = ["ACT", "DVE"]
        cap = MAX_CONSEC[last_ev["e"]] if last_ev["e"] else 9
        if last_ev["n"] >= cap and last_ev["e"] in cands:
            cands.remove(last_ev["e"])
        k = min(cands, key=lambda e: busy[e] + costs[e])
        busy[k] += costs[k]
        if k == last_ev["e"]:
            last_ev["n"] += 1
        else:
            last_ev["e"], last_ev["n"] = k, 1
        return k

    def charge(eng_name, cost):
        busy[eng_name] += cost

    with ExitStack() as ctx:
        tc = ctx.enter_context(tile.TileContext(nc))
        eng = {"ACT": nc.scalar, "DVE": nc.vector, "POOL": nc.gpsimd, "SP": nc.sync}

        def copy_on(e, out, in_):
            if e == "ACT":
                eng[e].copy(out, in_)
            else:
                eng[e].tensor_copy(out, in_)

        const_pool = ctx.enter_context(tc.tile_pool(name="const", bufs=1))
        xv_pool = ctx.enter_context(tc.tile_pool(name="xv", bufs=2))
        stage_pool = ctx.enter_context(tc.tile_pool(name="stage", bufs=2))
        pair_pool = ctx.enter_context(tc.tile_pool(name="pair", bufs=2))
        norm_pool = ctx.enter_context(tc.tile_pool(name="norm", bufs=2))
        psw_pool = ctx.enter_context(tc.tile_pool(name="psw", bufs=3, space="PSUM"))
        pvt_pool = ctx.enter_context(tc.tile_pool(name="pvt", bufs=1, space="PSUM"))
        psv_pool = ctx.enter_context(tc.tile_pool(name="psv", bufs=1, space="PSUM"))
        out_pool = ctx.enter_context(tc.tile_pool(name="outb", bufs=8))

        # identity: DMA-landed, staged through ACT (cast to fp16) so matmuls
        # only wait on ACT
        id_dma = const_pool.tile([P, P], f32)
        nc.sync.dma_start(out=id_dma, in_=id_in[:, :])
        identity = const_pool.tile([P, P], f16)
        nc.scalar.copy(identity, id_dma)
        charge("ACT", 300.0)
        # warmup matmuls: absorb the ACT(identity) wait and hold the PE
        # p-state ramp until the first real transposes arrive
        ps_warm = psv_pool.tile([Dh, 512], f32, tag="psv")
        for _ in range(10):
            nc.tensor.matmul(
                ps_warm[:, :P],
                lhsT=identity[:, :Dh],
                rhs=identity,
                start=True,
                stop=True,
            )

        gc = (nb * R) // ng  # (n, r) chunks per 512-row group = 4
        state = {}  # per-pair tiles
        # one-chunk store deferral per DMA queue against HOL blocking
        defer = {"SP": [], "POOL": []}

        def emit_store(e, o_, i_):
            defer[e].append((o_, i_))
            if len(defer[e]) > 1:
                oo, ii = defer[e].pop(0)
                eng[e].dma_start(out=oo, in_=ii)

        def flush_stores():
            for e in ("SP", "POOL"):
                for oo, ii in defer[e]:
                    eng[e].dma_start(out=oo, in_=ii)
                defer[e].clear()

        def prep_load_norm(p):
            """Load x[p] (512 B runs per partition), row-normalize, cast fp16."""
            s_dma = stage_pool.tile([Dh, Dh], f32, tag="sdma")
            nc.gpsimd.dma_start(out=s_dma, in_=s_in[p])
            charge("POOL", 500.0)
            s_sb = pair_pool.tile([Dh, Dh], f16, tag="s")
            nc.gpsimd.tensor_copy(s_sb[:], s_dma)
            charge("POOL", 120.0)

            v_sb = xv_pool.tile([P, nb * R, Dh], f32, tag="v")
            vsq = norm_pool.tile([P, nb * R, Dh], f32, tag="vsq")
            sumsq = norm_pool.tile([P, nb * R], f32, tag="ss")
            if p > 0:
                nc.sync.dma_start(
                    out=v_sb[:].rearrange("p (n r) d -> p n r d", r=R),
                    in_=x_in[p].rearrange("(n p r) d -> p n r d", p=P, r=R),
                )
                charge("SP", _LOAD_PAIR + 150.0)
            for g in range(ng):
                if p == 0:
                    ld = ("SP", "POOL", "ACT", "POOL")[g % 4]
                    eng[ld].dma_start(
                        out=v_sb[:, g * gc : (g + 1) * gc, :].rearrange(
                            "p (n r) d -> p n r d", r=R
                        ),
                        in_=x_in[p][g * 512 : (g + 1) * 512, :].rearrange(
                            "(n p r) d -> p n r d", p=P, r=R
                        ),
                    )
                    charge(ld, _LOAD_PAIR / ng + 150.0)
                sl3 = (slice(None), slice(g * gc, (g + 1) * gc))
                ne = nc.gpsimd
                ne.tensor_mul(
                    vsq[sl3 + (slice(None),)],
                    v_sb[sl3 + (slice(None),)],
                    v_sb[sl3 + (slice(None),)],
                )
                # tree-reduce along d (gpsimd cannot reduce the free axis
                # directly, and DVE -- the evac bottleneck -- should not)
                w = Dh
                while w > 1:
                    w //= 2
                    ne.tensor_add(
                        vsq[sl3 + (slice(0, w),)],
                        vsq[sl3 + (slice(0, w),)],
                        vsq[sl3 + (slice(w, 2 * w),)],
                    )
                ne.tensor_copy(
                    sumsq[:, g * gc : (g + 1) * gc], vsq[sl3 + (0,)]
                )
                charge("POOL", _NORM_GROUP + 450.0)
            nrm = norm_pool.tile([P, nb * R], f32, tag="nrm")
            rinv = norm_pool.tile([P, nb * R], f32, tag="rinv")
            v_hat = pair_pool.tile([P, nb * R, Dh], f16, tag="vhat")
            for g in range(ng):
                sl = slice(g * gc, (g + 1) * gc)
                nc.scalar.activation(
                    nrm[:, sl], sumsq[:, sl], mybir.ActivationFunctionType.Sqrt
                )
                charge("ACT", 200.0)
                nc.vector.reciprocal(rinv[:, sl], nrm[:, sl])
                charge("DVE", 80.0)
                rb = rinv[:, sl].unsqueeze(-1).broadcast_to((P, gc, Dh))
                nc.gpsimd.tensor_mul(v_hat[:, sl, :], v_sb[:, sl, :], rb)
                charge("POOL", _VHAT_GROUP)
            vt_sb = pair_pool.tile([Dh, t], f16, tag="vt")
            svt_sb = pair_pool.tile([Dh, t], f16, tag="svt")
            state[p] = {"s": s_sb, "vhat": v_hat, "vt": vt_sb, "svt": svt_sb}

        def prep_pe_gpair(p, gp):
            """PE-transpose groups 2*gp and 2*gp+1 of v_hat into vt (native
            block order, one amortized 1024-wide evacuation; the wedge rhs
            un-permutes with a strided read), then form each group's SvT
            slice. The all-fp16 straight copy hits DVE's 2x half-word mode.
            The very first group evacuates 512-wide so the pipeline fill
            does not wait for its sibling group's transposes."""
            st = state[p]
            ps_vt = pvt_pool.tile([Dh, 1024], f16, tag="pvt")
            for gg in range(2):
                g = gp * 2 + gg
                for j in range(gc):
                    nc.tensor.transpose(
                        ps_vt[:, gg * 512 + j * P : gg * 512 + (j + 1) * P],
                        st["vhat"][:, g * gc + j, :],
                        identity,
                    )
            e = pick(("ACT", "DVE"), _VT_EVAC)
            copy_on(
                e,
                st["vt"][:, gp * 1024 : (gp + 1) * 1024],
                ps_vt,
            )
            for gg in range(2):
                g = gp * 2 + gg
                ps_sv = psv_pool.tile([Dh, 512], f32, tag="psv")
                nc.tensor.matmul(
                    ps_sv,
                    lhsT=st["s"][:],
                    rhs=st["vt"][:, g * 512 : (g + 1) * 512],
                    start=True,
                    stop=True,
                )
                # rhs was block-major, so ps_sv is already block-ordered --
                # exactly the layout the wedge lhsT slices want
                e = pick(("ACT", "DVE"), _EVAC_512)
                copy_on(
                    e,
                    st["svt"][:, g * 512 : (g + 1) * 512],
                    ps_sv,
                )

        def wedge_block(p, n, h_range=None, ob=None):
            """One 256-row n-block: 4 [128, 1024] PSUM ring tiles; ACT/DVE
            alternate evacuations (fp16 cast); every [128, 1024] chunk is
            stored as its own fp16 DMA on SP or Pool (one chunk deferred)."""
            st = state[p]
            if ob is None:
                ob = out_pool.tile([P, R, t], f16, tag="ob")
            dst = out_d[p][n * 256 : (n + 1) * 256, :].rearrange(
                "(j r) c -> j r c", r=R
            )
            store = {k: _STORE_Q for k in ("SP", "POOL", "ACT")}
            last_blk = p == pairs - 1 and n == nb - 1
            for r in range(R):
                blk = n * R + r
                for h in h_range if h_range is not None else range(t // 1024):
                    fine = last_blk
                    ps_w = psw_pool.tile([P, 1024], f32, tag="psw")
                    for u in range(2):
                        nc.tensor.matmul(
                            ps_w[:, u * 512 : (u + 1) * 512],
                            lhsT=st["svt"][:, blk * P : (blk + 1) * P],
                            rhs=st["vt"][
                                :, (h * 2 + u) * 512 : (h * 2 + u + 1) * 512
                            ].rearrange("d (a r j) -> d a j r", a=2, r=R, j=P),
                            start=True,
                            stop=True,
                        )
                    if fine:
                        # pipeline drain: split the tail 512-wide over both
                        # evacuators and both store queues
                        for u in range(2):
                            lo = h * 1024 + u * 512
                            ev = pick_ev(_EVAC_512)
                            copy_on(
                                ev,
                                ob[:, r, lo : lo + 512],
                                ps_w[:, u * 512 : (u + 1) * 512],
                            )
                            e = pick(("SP", "POOL"), {k: 500.0 for k in ("SP", "POOL")})
                            eng[e].dma_start(
                                out=dst[:, r, lo : lo + 512],
                                in_=ob[:, r, lo : lo + 512],
                            )
                        continue
                    ev = pick_ev(_EVAC_1024)
                    copy_on(ev, ob[:, r, h * 1024 : (h + 1) * 1024], ps_w)
                    if last_blk:
                        # tail is latency-bound: rotate stores across all
                        # three DMA queues regardless of cumulative busy
                        e = ("SP", "POOL", "ACT")[(r * 2 + h) % 3]
                        charge(e, _STORE_Q)
                    else:
                        e = pick(("SP", "POOL"), store)
                    emit_store(
                        e,
                        dst[:, r, h * 1024 : (h + 1) * 1024],
                        ob[:, r, h * 1024 : (h + 1) * 1024],
                    )

        # software pipeline: pair p's wedge overlaps pair p+1's load/norm
        # (emitted first so DVE runs it early) and its transpose/Sv groups
        # (interleaved mid-wedge so the evacs drain before the wedge tail)
        prep_load_norm(0)
        for gp in range(ng // 2):
            prep_pe_gpair(0, gp)
        for p in range(pairs):
            if p + 1 < pairs:
                prep_load_norm(p + 1)
            for n in range(nb):
                if p == pairs - 1 and n == nb - 1:
                    flush_stores()
                wedge_block(p, n)
                if p + 1 < pairs and n - 3 in range(ng // 2):
                    prep_pe_gpair(p + 1, n - 3)
            state.pop(p)
        flush_stores()

    if spill:
        _spill_waits(nc)
    nc._balancer_busy = dict(busy)
    return nc


def _spill_waits(nc, multi_ok=("EventSemaphore",), max_keep=1):
    """Walrus encodes at most one sync-wait on Matmult (embedded weight load)
    and DMACopy; move extra waits onto a preceding same-engine EventSemaphore
    (which supports many waits). The engine sequencer processes instructions
    in order, so a preceding wait is semantically identical."""
    from concourse import mybir

    n_spilled = 0
    for f in nc.m.functions:
        for bb in f.blocks:
            il = bb.instructions
            out = []
            for inst in il:
                si = getattr(inst, "sync_info", None)
                waits = list((si.on_wait if si else None) or [])
                cap = 2 if inst.opcode in multi_ok else max_keep
                if len(waits) > cap:
                    moved, keep = waits[:-max_keep], waits[-max_keep:]
                    for k in range(0, len(moved), 2):
                        es = mybir.InstEventSemaphore(
                            name=f"{inst.name}-wspill{k}",
                            engine=inst.engine,
                            ins=[],
                            outs=[],
                            sync_info=mybir.SyncInfo(
                                on_wait=moved[k : k + 2], on_update=[]
                            ),
                        )
                        out.append(es)
                    inst.sync_info = mybir.SyncInfo(
                        on_wait=keep, on_update=list(si.on_update or [])
                    )
                    n_spilled += 1
                out.append(inst)
            il[:] = out
    return n_spilled


def _import_concourse():
    try:
        import concourse  # noqa: F401
    except ImportError:
        import sys

        for p in ("/opt/trn_rl_repo", "/root/.axon_site/_ro/trn_rl_repo"):
            if p not in sys.path:
                sys.path.insert(0, p)


def _ensure_device_backend():
    """If the process pinned JAX_PLATFORMS to cpu, lift the pin so the
    NeuronCores (axon platform) are reachable for the kernel run."""
    import os

    plats = os.environ.get("JAX_PLATFORMS", "")
    if plats and "axon" not in plats and "neuron" not in plats:
        os.environ["JAX_PLATFORMS"] = ""
        try:
            import jax

            jax.extend.backend.clear_backends()
        except Exception:
            pass


def kernel(x, A, window_size=None):
    _import_concourse()
    _ensure_device_backend()
    from concourse.bass_utils import run_bass_kernel_spmd

    x = np.ascontiguousarray(x, dtype=np.float32)
    A = np.ascontiguousarray(A, dtype=np.float32)
    assert x.shape == (B, T, D) and A.shape == (H, Dh, Dh)

    nc = _COMPILED.get("nc")
    if nc is None:
        nc = _build_nc()
        _COMPILED["nc"] = nc

    # x[b, t, h*64:(h+1)*64] per (b,h) pair; pair index bh = b*H + h.
    xv = x.reshape(B, T, H, Dh).transpose(0, 2, 1, 3).reshape(B * H, T, Dh)
    S = (A - np.swapaxes(A, -1, -2)).astype(np.float32)  # replicated with heads
    S_all = np.tile(S, (B, 1, 1))
    ident = np.eye(P, dtype=np.float32)
    in_maps = []
    for c in range(N_CORES):
        sl = slice(c * PAIRS, (c + 1) * PAIRS)
        in_maps.append(
            {
                "x": np.ascontiguousarray(xv[sl]),
                "s": np.ascontiguousarray(S_all[sl]),
                "ident": ident,
            }
        )
    res = run_bass_kernel_spmd(nc, in_maps, list(range(N_CORES)), trace=TRACE)
    global LAST_RESULT
    LAST_RESULT = res
    outs = [np.asarray(res.results[c]["out"]) for c in range(N_CORES)]
    full = np.concatenate(outs, axis=0).reshape(B, H, T, T).astype(np.float32)
    return full


# revision 56
# speedup vs baseline: 1.0057x; 1.0057x over previous
"""Trainium2 Bass kernel for nn_DirectionalWedgeBias.

Computes, per (batch b, head h):
    v      = x[b].reshape(T, H, Dh)[:, h, :]          # [T, Dh]
    v_hat  = v / max(||v||_2, eps)  (row-wise)
    S      = A[h] - A[h]^T                            # [Dh, Dh]
    wedge  = (v_hat @ S) @ v_hat^T                    # [T, T]

Full shapes: x [2, 2048, 1024] f32, A [16, 64, 64] f32 -> out [2, 16, 2048, 2048] f32.

Sharding: 32 independent (b, h) pairs split 4-per-core across 8 NeuronCores
(data + head parallel; the tiny skew-symmetric S is replicated/sliced with the
heads). Host pre-slices x into per-core [4, T, Dh] blocks, forms S = A - A^T,
and re-stacks the per-core [4, T, T] results.

Per-core dataflow (Tile framework), fp16 end to end (rel err ~5e-4 vs the
2e-2 gate; f32 is kept only for the row norms). The kernel is bound by
PSUM->SBUF evacuation: matmuls may only write f32 PSUM, and only ACT and DVE
may read PSUM (the BIR verifier rejects GPSIMD PSUM access), so those two
engines are reserved for evacuation and everything else is pushed elsewhere:
  - x loads use a 2-rows-per-partition interleave (t = n*256 + 2*part + r) so
    each partition receives contiguous 512 B runs (full-rate descriptors
    instead of the <512 B read-modify-write class)
  - row-normalize per 512-row group: square + tree-reduce + v_hat multiply
    run on Pool/gpsimd (SBUF-only ops are legal there; gpsimd cannot reduce
    the free axis so the reduction is a log2 tensor_add tree), ACT does the
    sqrt, DVE only the tiny reciprocal; v_hat is written as fp16
  - PE-transpose v_hat chunks into vt [64, T] fp16, kept in native block
    order: the transpose PSUM tile is fp16, so the straight (non-scattering)
    copy qualifies for DVE's 2x half-word mode; the wedge rhs un-permutes
    via a strided read at no modeled cost. SvT = S @ vT (block-order rhs)
    lands block-ordered, so wedge lhsT slices stay contiguous
  - wedge: two fp16 matmuls per [128, 1024] PSUM tile in a 3-deep ring (6 of
    the 8 banks; 1 bank each for the transpose and SvT tiles); ACT and DVE
    strictly alternate ring slots so the ring never waits twice on one
    queue, and each [128, 1024] chunk is cast to fp16 into the staging tile
  - stores are fp16 (half the f32 DMA bytes) and each chunk is its own DMA
    (free in the cost model -- DMA cost is purely bytes-proportional above
    the 500 ns floor), load-balanced over SP and Pool, with a one-chunk
    deferral per queue so a store whose evacuation lags never head-of-line
    blocks the queue; the host widens fp16 back to f32
  - a greedy balancer assigns every evacuation/store/load by modeled ns cost
  - walrus encodes at most ONE semaphore wait on most instructions, so
    `_spill_waits` post-processes the Tile-scheduled BIR, hoisting excess
    waits onto preceding same-engine EventSemaphores

Cost-model (CoreSim) per-core time: ~95.1 us (engine busy: DVE ~84, ACT ~83,
SP ~64, Pool ~55, PE ~63 -- evacuation-bound; the legal floor for this
dataflow is ~93 us). Verified end to end through neuronxcc + the 8-core axon
run at rel err 4.6e-4.
"""

import numpy as np

B = 2
T = 2048
D = 1024
H = 16
Dh = 64
N_CORES = 8
PAIRS = (B * H) // N_CORES  # 4 per core
P = 128  # SBUF partitions
NB = T // 256  # 8 n-blocks (256 rows) per pair
R = 2  # t-rows per partition within an n-block

_COMPILED = {}

# test-harness knobs (default off; harness calls kernel() with these untouched)
TRACE = False
LAST_RESULT = None

# modeled per-instruction costs (ns) for the greedy engine balancer.
# Only ACT and DVE can read PSUM (the BIR verifier rejects GPSIMD PSUM
# access), so they evacuate; Pool does the SBUF-side norm math and shares
# stores/loads with SP.
_EVAC_1024 = {"ACT": 1100.0, "DVE": 1260.0}
_VT_EVAC = {"ACT": 1100.0, "DVE": 720.0}  # fp16 PSUM src: DVE gets 2x mode
_EVAC_512 = {"ACT": 670.0, "DVE": 900.0}
_STORE_Q = 850.0  # [128, 1024] fp16 chunk = 2048 B/partition
_LOAD_PAIR = 1729.0
_NORM_GROUP = 490.0  # vsq + reduce per 512-row group ([128, 4, 64]) on Pool
_VHAT_GROUP = 270.0
MAX_CONSEC = {"ACT": 1, "DVE": 1}  # max consecutive ring slots per engine


def _build_nc(pairs=PAIRS, t=T, spill=True):
    _import_concourse()
    from contextlib import ExitStack

    import concourse.bass as bass
    import concourse.tile as tile
    from concourse import mybir

    f32 = mybir.dt.float32
    f16 = mybir.dt.float16
    nb = t // 256  # n-blocks per pair
    ng = t // 512  # 512-row load/norm groups per pair

    nc = bass.Bass()
    x_in = nc.declare_dram_parameter("x", [pairs, t, Dh], f32, isOutput=False)
    s_in = nc.declare_dram_parameter("s", [pairs, Dh, Dh], f32, isOutput=False)
    id_in = nc.declare_dram_parameter("ident", [P, P], f32, isOutput=False)
    out_d = nc.declare_dram_parameter("out", [pairs, t, t], f16, isOutput=True)

    busy = {"ACT": 0.0, "DVE": 0.0, "POOL": 0.0, "SP": 0.0}
    last_ev = {"e": None, "n": 0}

    def pick(cands, costs):
        k = min(cands, key=lambda e: busy[e] + costs[e])
        busy[k] += costs[k]
        return k

    def pick_ev(costs):
        # balanced ACT/DVE evac choice; cap consecutive ring slots per engine
        # (ACT is the faster evacuator so it may take short bursts, DVE never
        # repeats) so the PSUM ring always drains through both queues
        cands = ["ACT", "DVE"]
        cap = MAX_CONSEC[last_ev["e"]] if last_ev["e"] else 9
        if last_ev["n"] >= cap and last_ev["e"] in cands:
            cands.remove(last_ev["e"])
        k = min(cands, key=lambda e: busy[e] + costs[e])
        busy[k] += costs[k]
        if k == last_ev["e"]:
            last_ev["n"] += 1
        else:
            last_ev["e"], last_ev["n"] = k, 1
        return k

    def charge(eng_name, cost):
        busy[eng_name] += cost

    with ExitStack() as ctx:
        tc = ctx.enter_context(tile.TileContext(nc))
        eng = {"ACT": nc.scalar, "DVE": nc.vector, "POOL": nc.gpsimd, "SP": nc.sync}

        def copy_on(e, out, in_):
            if e == "ACT":
                eng[e].copy(out, in_)
            else:
                eng[e].tensor_copy(out, in_)

        const_pool = ctx.enter_context(tc.tile_pool(name="const", bufs=1))
        xv_pool = ctx.enter_context(tc.tile_pool(name="xv", bufs=2))
        stage_pool = ctx.enter_context(tc.tile_pool(name="stage", bufs=2))
        pair_pool = ctx.enter_context(tc.tile_pool(name="pair", bufs=2))
        norm_pool = ctx.enter_context(tc.tile_pool(name="norm", bufs=2))
        psw_pool = ctx.enter_context(tc.tile_pool(name="psw", bufs=3, space="PSUM"))
        pvt_pool = ctx.enter_context(tc.tile_pool(name="pvt", bufs=1, space="PSUM"))
        psv_pool = ctx.enter_context(tc.tile_pool(name="psv", bufs=1, space="PSUM"))
        out_pool = ctx.enter_context(tc.tile_pool(name="outb", bufs=8))

        # identity: DMA-landed, staged through ACT (cast to fp16) so matmuls
        # only wait on ACT
        id_dma = const_pool.tile([P, P], f32)
        nc.sync.dma_start(out=id_dma, in_=id_in[:, :])
        identity = const_pool.tile([P, P], f16)
        nc.scalar.copy(identity, id_dma)
        charge("ACT", 300.0)
        # warmup matmuls: absorb the ACT(identity) wait and hold the PE
        # p-state ramp until the first real transposes arrive
        ps_warm = psv_pool.tile([Dh, 512], f32, tag="psv")
        for _ in range(10):
            nc.tensor.matmul(
                ps_warm[:, :P],
                lhsT=identity[:, :Dh],
                rhs=identity,
                start=True,
                stop=True,
            )

        gc = (nb * R) // ng  # (n, r) chunks per 512-row group = 4
        state = {}  # per-pair tiles
        # one-chunk store deferral per DMA queue against HOL blocking
        defer = {"SP": [], "POOL": []}

        def emit_store(e, o_, i_):
            defer[e].append((o_, i_))
            if len(defer[e]) > 1:
                oo, ii = defer[e].pop(0)
                eng[e].dma_start(out=oo, in_=ii)

        def flush_stores():
            for e in ("SP", "POOL"):
                for oo, ii in defer[e]:
                    eng[e].dma_start(out=oo, in_=ii)
                defer[e].clear()

        def prep_load_norm(p):
            """Load x[p] (512 B runs per partition), row-normalize, cast fp16."""
            s_dma = stage_pool.tile([Dh, Dh], f32, tag="sdma")
            nc.gpsimd.dma_start(out=s_dma, in_=s_in[p])
            charge("POOL", 500.0)
            s_sb = pair_pool.tile([Dh, Dh], f16, tag="s")
            nc.gpsimd.tensor_copy(s_sb[:], s_dma)
            charge("POOL", 120.0)

            v_sb = xv_pool.tile([P, nb * R, Dh], f32, tag="v")
            vsq = norm_pool.tile([P, nb * R, Dh], f32, tag="vsq")
            sumsq = norm_pool.tile([P, nb * R], f32, tag="ss")
            if p > 0:
                nc.sync.dma_start(
                    out=v_sb[:].rearrange("p (n r) d -> p n r d", r=R),
                    in_=x_in[p].rearrange("(n p r) d -> p n r d", p=P, r=R),
                )
                charge("SP", _LOAD_PAIR + 150.0)
            for g in range(ng):
                if p == 0:
                    ld = ("SP", "POOL", "ACT", "POOL")[g % 4]
                    eng[ld].dma_start(
                        out=v_sb[:, g * gc : (g + 1) * gc, :].rearrange(
                            "p (n r) d -> p n r d", r=R
                        ),
                        in_=x_in[p][g * 512 : (g + 1) * 512, :].rearrange(
                            "(n p r) d -> p n r d", p=P, r=R
                        ),
                    )
                    charge(ld, _LOAD_PAIR / ng + 150.0)
                sl3 = (slice(None), slice(g * gc, (g + 1) * gc))
                ne = nc.gpsimd
                ne.tensor_mul(
                    vsq[sl3 + (slice(None),)],
                    v_sb[sl3 + (slice(None),)],
                    v_sb[sl3 + (slice(None),)],
                )
                # tree-reduce along d (gpsimd cannot reduce the free axis
                # directly, and DVE -- the evac bottleneck -- should not)
                w = Dh
                while w > 1:
                    w //= 2
                    ne.tensor_add(
                        vsq[sl3 + (slice(0, w),)],
                        vsq[sl3 + (slice(0, w),)],
                        vsq[sl3 + (slice(w, 2 * w),)],
                    )
                ne.tensor_copy(
                    sumsq[:, g * gc : (g + 1) * gc], vsq[sl3 + (0,)]
                )
                charge("POOL", _NORM_GROUP + 450.0)
            nrm = norm_pool.tile([P, nb * R], f32, tag="nrm")
            rinv = norm_pool.tile([P, nb * R], f32, tag="rinv")
            v_hat = pair_pool.tile([P, nb * R, Dh], f16, tag="vhat")
            for g in range(ng):
                sl = slice(g * gc, (g + 1) * gc)
                nc.scalar.activation(
                    nrm[:, sl], sumsq[:, sl], mybir.ActivationFunctionType.Sqrt
                )
                charge("ACT", 200.0)
                nc.vector.reciprocal(rinv[:, sl], nrm[:, sl])
                charge("DVE", 80.0)
                rb = rinv[:, sl].unsqueeze(-1).broadcast_to((P, gc, Dh))
                nc.gpsimd.tensor_mul(v_hat[:, sl, :], v_sb[:, sl, :], rb)
                charge("POOL", _VHAT_GROUP)
            vt_sb = pair_pool.tile([Dh, t], f16, tag="vt")
            svt_sb = pair_pool.tile([Dh, t], f16, tag="svt")
            state[p] = {"s": s_sb, "vhat": v_hat, "vt": vt_sb, "svt": svt_sb}

        def prep_pe_gpair(p, gp):
            """PE-transpose groups 2*gp and 2*gp+1 of v_hat into vt (native
            block order, one amortized 1024-wide evacuation; the wedge rhs
            un-permutes with a strided read), then form each group's SvT
            slice. The all-fp16 straight copy hits DVE's 2x half-word mode.
            The very first group evacuates 512-wide so the pipeline fill
            does not wait for its sibling group's transposes."""
            st = state[p]
            ps_vt = pvt_pool.tile([Dh, 1024], f16, tag="pvt")
            for gg in range(2):
                g = gp * 2 + gg
                for j in range(gc):
                    nc.tensor.transpose(
                        ps_vt[:, gg * 512 + j * P : gg * 512 + (j + 1) * P],
                        st["vhat"][:, g * gc + j, :],
                        identity,
                    )
            e = pick(("ACT", "DVE"), _VT_EVAC)
            copy_on(
                e,
                st["vt"][:, gp * 1024 : (gp + 1) * 1024],
                ps_vt,
            )
            for gg in range(2):
                g = gp * 2 + gg
                ps_sv = psv_pool.tile([Dh, 512], f32, tag="psv")
                nc.tensor.matmul(
                    ps_sv,
                    lhsT=st["s"][:],
                    rhs=st["vt"][:, g * 512 : (g + 1) * 512],
                    start=True,
                    stop=True,
                )
                # rhs was block-major, so ps_sv is already block-ordered --
                # exactly the layout the wedge lhsT slices want
                e = pick(("ACT", "DVE"), _EVAC_512)
                copy_on(
                    e,
                    st["svt"][:, g * 512 : (g + 1) * 512],
                    ps_sv,
                )

        def wedge_block(p, n, h_range=None, ob=None):
            """One 256-row n-block: 4 [128, 1024] PSUM ring tiles; ACT/DVE
            alternate evacuations (fp16 cast); every [128, 1024] chunk is
            stored as its own fp16 DMA on SP or Pool (one chunk deferred)."""
            st = state[p]
            if ob is None:
                ob = out_pool.tile([P, R, t], f16, tag="ob")
            dst = out_d[p][n * 256 : (n + 1) * 256, :].rearrange(
                "(j r) c -> j r c", r=R
            )
            store = {k: _STORE_Q for k in ("SP", "POOL", "ACT")}
            last_blk = p == pairs - 1 and n == nb - 1
            for r in range(R):
                blk = n * R + r
                for h in h_range if h_range is not None else range(t // 1024):
                    fine = last_blk
                    ps_w = psw_pool.tile([P, 1024], f32, tag="psw")
                    for u in range(2):
                        nc.tensor.matmul(
                            ps_w[:, u * 512 : (u + 1) * 512],
                            lhsT=st["svt"][:, blk * P : (blk + 1) * P],
                            rhs=st["vt"][
                                :, (h * 2 + u) * 512 : (h * 2 + u + 1) * 512
                            ].rearrange("d (a r j) -> d a j r", a=2, r=R, j=P),
                            start=True,
                            stop=True,
                        )
                    if fine:
                        # pipeline drain: split the tail 512-wide over both
                        # evacuators and both store queues
                        for u in range(2):
                            lo = h * 1024 + u * 512
                            ev = pick_ev(_EVAC_512)
                            copy_on(
                                ev,
                                ob[:, r, lo : lo + 512],
                                ps_w[:, u * 512 : (u + 1) * 512],
                            )
                            e = pick(("SP", "POOL"), {k: 500.0 for k in ("SP", "POOL")})
                            eng[e].dma_start(
                                out=dst[:, r, lo : lo + 512],
                                in_=ob[:, r, lo : lo + 512],
                            )
                        continue
                    ev = pick_ev(_EVAC_1024)
                    copy_on(ev, ob[:, r, h * 1024 : (h + 1) * 1024], ps_w)
                    if last_blk:
                        # tail is latency-bound: rotate stores across all
                        # three DMA queues regardless of cumulative busy
                        e = ("SP", "POOL", "ACT")[(r * 2 + h) % 3]
                        charge(e, _STORE_Q)
                    else:
                        e = pick(("SP", "POOL"), store)
                    emit_store(
                        e,
                        dst[:, r, h * 1024 : (h + 1) * 1024],
                        ob[:, r, h * 1024 : (h + 1) * 1024],
                    )

        # software pipeline: pair p's wedge overlaps pair p+1's load/norm
        # (emitted first so DVE runs it early) and its transpose/Sv groups
        # (interleaved mid-wedge so the evacs drain before the wedge tail)
        prep_load_norm(0)
        for gp in range(ng // 2):
            prep_pe_gpair(0, gp)
        for p in range(pairs):
            if p + 1 < pairs:
                prep_load_norm(p + 1)
            for n in range(nb):
                if p == pairs - 1 and n == nb - 1:
                    flush_stores()
                wedge_block(p, n)
                if p + 1 < pairs and n - 3 in range(ng // 2):
                    prep_pe_gpair(p + 1, n - 3)
            state.pop(p)
        flush_stores()

    if spill:
        _spill_waits(nc)
    nc._balancer_busy = dict(busy)
    return nc


def _spill_waits(nc, multi_ok=("EventSemaphore",), max_keep=1):
    """Walrus encodes at most one sync-wait on Matmult (embedded weight load)
    and DMACopy; move extra waits onto a preceding same-engine EventSemaphore
    (which supports many waits). The engine sequencer processes instructions
    in order, so a preceding wait is semantically identical."""
    from concourse import mybir

    n_spilled = 0
    for f in nc.m.functions:
        for bb in f.blocks:
            il = bb.instructions
            out = []
            for inst in il:
                si = getattr(inst, "sync_info", None)
                waits = list((si.on_wait if si else None) or [])
                cap = 2 if inst.opcode in multi_ok else max_keep
                if len(waits) > cap:
                    moved, keep = waits[:-max_keep], waits[-max_keep:]
                    for k in range(0, len(moved), 2):
                        es = mybir.InstEventSemaphore(
                            name=f"{inst.name}-wspill{k}",
                            engine=inst.engine,
                            ins=[],
                            outs=[],
                            sync_info=mybir.SyncInfo(
                                on_wait=moved[k : k + 2], on_update=[]
                            ),
                        )
                        out.append(es)
                    inst.sync_info = mybir.SyncInfo(
                        on_wait=keep, on_update=list(si.on_update or [])
                    )
                    n_spilled += 1
                out.append(inst)
            il[:] = out
    return n_spilled


def _import_concourse():
    try:
        import concourse  # noqa: F401
    except ImportError:
        import sys

        for p in ("/opt/trn_rl_repo", "/root/.axon_site/_ro/trn_rl_repo"):
            if p not in sys.path:
                sys.path.insert(0, p)


def _ensure_device_backend():
    """If the process pinned JAX_PLATFORMS to cpu, lift the pin so the
    NeuronCores (axon platform) are reachable for the kernel run."""
    import os

    plats = os.environ.get("JAX_PLATFORMS", "")
    if plats and "axon" not in plats and "neuron" not in plats:
        os.environ["JAX_PLATFORMS"] = ""
        try:
            import jax

            jax.extend.backend.clear_backends()
        except Exception:
            pass


def kernel(x, A, window_size=None):
    _import_concourse()
    _ensure_device_backend()
    from concourse.bass_utils import run_bass_kernel_spmd

    x = np.ascontiguousarray(x, dtype=np.float32)
    A = np.ascontiguousarray(A, dtype=np.float32)
    assert x.shape == (B, T, D) and A.shape == (H, Dh, Dh)

    nc = _COMPILED.get("nc")
    if nc is None:
        nc = _build_nc()
        _COMPILED["nc"] = nc

    # x[b, t, h*64:(h+1)*64] per (b,h) pair; pair index bh = b*H + h.
    xv = x.reshape(B, T, H, Dh).transpose(0, 2, 1, 3).reshape(B * H, T, Dh)
    S = (A - np.swapaxes(A, -1, -2)).astype(np.float32)  # replicated with heads
    S_all = np.tile(S, (B, 1, 1))
    ident = np.eye(P, dtype=np.float32)
    in_maps = []
    for c in range(N_CORES):
        sl = slice(c * PAIRS, (c + 1) * PAIRS)
        in_maps.append(
            {
                "x": np.ascontiguousarray(xv[sl]),
                "s": np.ascontiguousarray(S_all[sl]),
                "ident": ident,
            }
        )
    res = run_bass_kernel_spmd(nc, in_maps, list(range(N_CORES)), trace=TRACE)
    global LAST_RESULT
    LAST_RESULT = res
    outs = [np.asarray(res.results[c]["out"]) for c in range(N_CORES)]
    full = np.concatenate(outs, axis=0).reshape(B, H, T, T).astype(np.float32)
    return full
